# revision 27
# baseline (speedup 1.0000x reference)
"""Trainium2 Bass kernel for DictionaryLearning-kSVD (OMP sparse coding / VQ).

Contract: kernel(**inputs) takes FULL inputs (z_e (64,64,32,32) f32,
dictionary (512,64) f32) and returns the FULL outputs matching reference():
(recon_loss, z_st, perplexity, rep).

Strategy: data-parallel over the flattened sample axis N = 64*32*32 = 65536
across 8 NeuronCores (8192 samples/core).  Per core, OMP runs as a
lockstep-batched MGS (modified Gram-Schmidt) recurrence:

  alpha lives in PSUM, updated by accumulating matmuls
      alpha <- alpha - (ytil * u_t) @ D^T
  selection per step via DVE tensor_tensor_reduce (abs-max) + max_index
  dictionary row gather via GPSIMD ap_gather on the transposed dictionary
  final coefficients via batched back-substitution of the unit-triangular
  MGS basis matrix; rep rows assembled with GPSIMD local_scatter (hi/lo
  bf16 split for exact f32).

Self-contained: hardcodes all shapes; no sibling imports.
"""

import numpy as np

import concourse.bass as bass
import concourse.bacc as bacc
import concourse.mybir as mybir
import concourse.tile as tile
from concourse import library_config
from concourse.bass_utils import run_bass_kernel_spmd

F32 = mybir.dt.float32
F32R = mybir.dt.float32r
U16 = mybir.dt.uint16
I16 = mybir.dt.int16
BF16 = mybir.dt.bfloat16
ALU = mybir.AluOpType
AXL = mybir.AxisListType
ACT = mybir.ActivationFunctionType

B, CH, HW = 64, 64, 1024          # z_e: (B, CH, 32, 32) -> HW = 1024
K, L, P = 512, 8, 128             # atoms, sparsity, partition tile
NCORES = 8
BPC = B // NCORES                 # 8 batches per core
NPC = BPC * HW                    # 8192 samples per core
C = 4                             # tiles per lockstep group
NT_FULL = NPC // P                # 64 tiles per core
BETA, EPS = 0.25, 1e-10


def _build_kernel(tc, z_ap, dt_ap, idf_ap, wsel_ap, rep_ap, zst_ap, part_ap,
                  nt, use_f32r):
    nc = tc.nc
    ngrp = nt // C
    mmdt = F32R if use_f32r else F32

    def mmap(ap):
        return ap.bitcast(F32R) if use_f32r else ap

    from contextlib import ExitStack
    ctx = ExitStack()
    const = ctx.enter_context(tc.tile_pool(name="const", bufs=1))
    gpool = ctx.enter_context(tc.tile_pool(name="grp", bufs=2))
    spool = ctx.enter_context(tc.tile_pool(name="step", bufs=2))
    accp = ctx.enter_context(tc.tile_pool(name="acc", bufs=1))
    ps_a = ctx.enter_context(tc.tile_pool(name="psA", bufs=1, space="PSUM"))
    ps_t = ctx.enter_context(tc.tile_pool(name="psT", bufs=3, space="PSUM"))
    ps_d = ctx.enter_context(tc.tile_pool(name="psD", bufs=1, space="PSUM"))

    # ---- constants ----
    DT = const.tile([64, K], F32)           # dictionary^T
    nc.sync.dma_start(DT[:], dt_ap[:, :])
    IDF = const.tile([P, P], F32)           # identity for PE transposes
    nc.sync.dma_start(IDF[:], idf_ap[:, :])
    WSEL = const.tile([P, 8 * 64], F32)     # wrap-selector matrices
    nc.sync.dma_start(WSEL[:], wsel_ap[:, :])
    ONES = const.tile([P, 1], F32)
    nc.vector.memset(ONES[:], 1.0)
    ZERO = const.tile([P, 1], F32)
    nc.vector.memset(ZERO[:], 0.0)

    ALLJ = accp.tile([P, nt, L], U16)
    ALLC = accp.tile([P, nt, L], F32)
    MSEA = accp.tile([P, C], F32)
    nc.vector.memset(MSEA[:], 0.0)

    lib1 = nc.gpsimd.load_library(library_config.ap_gather)
    gathers = []

    # ================= phase 1: selection + solve =================
    for g in range(ngrp):
        b, half = g // 2, g % 2
        off = half * (C * P)

        XT = gpool.tile([64, C * P], F32, tag="XT")
        nc.sync.dma_start(XT[:], z_ap[b, :, off:off + C * P])

        # sample-major X via PE transposes
        xps = ps_t.tile([P, C * 64], F32, tag="tps")
        for i in range(C):
            nc.tensor.transpose(xps[:, i * 64:(i + 1) * 64],
                                XT[:, i * P:(i + 1) * P], IDF[:64, :64])
        Xg = gpool.tile([P, C, 64], F32, tag="Xg")
        nc.scalar.copy(Xg[:], xps[:].rearrange("p (c d) -> p c d", c=C))

        # alpha0 = X @ D^T  (PSUM, one bank per tile)
        alpha = ps_a.tile([P, C * K], F32, tag="alpha")
        for i in range(C):
            nc.tensor.matmul(alpha[:, i * K:(i + 1) * K],
                             mmap(XT[:, i * P:(i + 1) * P]), mmap(DT[:]),
                             start=True, stop=False, skip_group_check=True)

        Ug = gpool.tile([P, L, C, 64], F32, tag="Ug")
        Bg = gpool.tile([P, L, L, C], F32, tag="Bg")      # (t, m, c)
        NUg = gpool.tile([P, L, C], F32, tag="NUg")
        YTg = gpool.tile([P, L, C], F32, tag="YTg")

        for t in range(L):
            # -- selection --
            absa = spool.tile([P, C, K], F32, tag="absa")
            vmax = spool.tile([P, C], F32, tag="vmax")
            for i in range(C):
                nc.scalar.activation(absa[:, i, :],
                                     alpha[:, i * K:(i + 1) * K], ACT.Abs)
            nc.vector.tensor_reduce(vmax[:], absa[:], axis=AXL.X, op=ALU.max)
            idx8 = spool.tile([P, C, 8], U16, tag="idx8")
            for i in range(C):
                nc.vector.max_index(idx8[:, i, :],
                                    vmax[:, i:i + 1].to_broadcast([P, 8]),
                                    absa[:, i, :])
            nc.vector.tensor_copy(ALLJ[:, g * C:(g + 1) * C, t], idx8[:, :, 0])
            jf = spool.tile([P, C], F32, tag="jf")
            nc.vector.tensor_copy(jf[:], idx8[:, :, 0])

            # -- index wrap for ap_gather via constant selector matmuls:
            #    idxw[16g+q, c, s] = j[16s+q, c]
            jwps = ps_t.tile([64, 8, C], F32, tag="tps")
            for s in range(8):
                nc.tensor.matmul(jwps[:, s, :],
                                 WSEL[:, s * 64:(s + 1) * 64], jf[:],
                                 start=True, stop=True, skip_group_check=True)
            idxwf = spool.tile([64, C, 8], F32, tag="idxwf")
            nc.scalar.copy(idxwf[:], jwps[:].transpose([0, 2, 1]))
            idxw = spool.tile([64, C, 8], I16, tag="idxw")
            nc.vector.tensor_copy(idxw[:], idxwf[:])

            # -- gather dictionary rows (transposed): djT[d, s] = D^T[d, j_s]
            djT = spool.tile([64, C, P], F32, tag="djT")
            for i in range(C):
                g_inst = nc.gpsimd.ap_gather(
                    out_ap=djT[:, i, :].unsqueeze(2),
                    in_ap=DT[:].unsqueeze(2),
                    idxs_ap=idxw[:, i, :],
                    channels=64, num_elems=K, d=1, num_idxs=P)
                bass._add_dep_helper(g_inst.ins, lib1.ins, sync=True,
                                     reason="gather after lib1")
                gathers.append(g_inst)
            dj = ps_d.tile([P, C, 64], F32, tag="dj")
            for i in range(C):
                nc.tensor.transpose(dj[:, i, :], djT[:, i, :], IDF[:64, :64])

            # -- MGS orthogonalization --
            if t > 0:
                scr1 = spool.tile([P, L, C, 64], F32, tag="scr1")
                nc.vector.tensor_mul(
                    scr1[:, :t, :, :], Ug[:, :t, :, :],
                    dj[:].unsqueeze(1).to_broadcast([P, t, C, 64]))
                wtil = spool.tile([P, L, C], F32, tag="wtil")
                nc.vector.tensor_reduce(wtil[:, :t, :], scr1[:, :t, :, :],
                                        axis=AXL.X, op=ALU.add)
                wbar = spool.tile([P, L, C], F32, tag="wbar")
                nc.vector.tensor_mul(wbar[:, :t, :], wtil[:, :t, :],
                                     NUg[:, :t, :])
                nc.vector.tensor_copy(Bg[:, t, :t, :], wbar[:, :t, :])
                scr2 = spool.tile([P, C, 64, L], F32, tag="scr2")
                nc.vector.tensor_mul(
                    scr2[:, :, :, :t],
                    Ug[:, :t, :, :].transpose([0, 2, 3, 1]),
                    wbar[:, :t, :].transpose([0, 2, 1]).unsqueeze(2)
                        .to_broadcast([P, C, 64, t]))
                proj = spool.tile([P, C, 64], F32, tag="proj")
                nc.vector.tensor_reduce(proj[:], scr2[:, :, :, :t],
                                        axis=AXL.X, op=ALU.add)
                nc.vector.tensor_sub(Ug[:, t, :, :], dj[:], proj[:])
            else:
                nc.vector.tensor_copy(Ug[:, 0, :, :], dj[:])

            uscr = spool.tile([P, C, 64], F32, tag="uscr")
            n2v = spool.tile([P, C], F32, tag="n2v")
            nc.vector.tensor_mul(uscr[:], Ug[:, t, :, :], Ug[:, t, :, :])
            nc.vector.tensor_reduce(n2v[:], uscr[:], axis=AXL.X, op=ALU.add)
            yv = spool.tile([P, C], F32, tag="yv")
            nc.vector.tensor_mul(uscr[:], Ug[:, t, :, :], Xg[:])
            nc.vector.tensor_reduce(yv[:], uscr[:], axis=AXL.X, op=ALU.add)
            nc.vector.reciprocal(NUg[:, t, :], n2v[:])
            nc.vector.tensor_mul(YTg[:, t, :], yv[:], NUg[:, t, :])

            # -- alpha update: alpha -= ytil * (u @ D^T) --
            ytn = spool.tile([P, C], F32, tag="ytn")
            nc.vector.tensor_scalar(ytn[:], YTg[:, t, :], -1.0, None,
                                    op0=ALU.mult)
            utl = spool.tile([P, C, 64], F32, tag="utl")
            nc.vector.tensor_mul(utl[:], Ug[:, t, :, :],
                                 ytn[:].unsqueeze(2).to_broadcast([P, C, 64]))
            uT = ps_t.tile([64, C * P], F32, tag="tps")
            for i in range(C):
                nc.tensor.transpose(uT[:, i * P:(i + 1) * P], utl[:, i, :],
                                    IDF[:, :])
            uTs = spool.tile([64, C * P], F32, tag="uTs")
            nc.scalar.copy(uTs[:], uT[:])
            for i in range(C):
                nc.tensor.matmul(alpha[:, i * K:(i + 1) * K],
                                 mmap(uTs[:, i * P:(i + 1) * P]), mmap(DT[:]),
                                 start=False, stop=(t == L - 1),
                                 skip_group_check=True)

        # ---- group tail: backsolve, rep coefs, recon, z_st, loss partials
        cg = gpool.tile([P, L, C], F32, tag="cg")
        nc.vector.tensor_copy(cg[:], YTg[:])
        bscr = spool.tile([P, C, L], F32, tag="bscr")
        red = spool.tile([P, C], F32, tag="red")
        for tt in range(L - 2, -1, -1):
            n = L - 1 - tt
            nc.vector.tensor_mul(bscr[:, :, :n],
                                 Bg[:, tt + 1:, tt, :].transpose([0, 2, 1]),
                                 cg[:, tt + 1:, :].transpose([0, 2, 1]))
            nc.vector.tensor_reduce(red[:], bscr[:, :, :n], axis=AXL.X,
                                    op=ALU.add)
            nc.vector.tensor_sub(cg[:, tt, :], cg[:, tt, :],
                                 red[:].rearrange("p c -> p c"))

        # 1/||x|| with one Newton refinement
        xx = spool.tile([P, C, 64], F32, tag="uscr")
        n2x = spool.tile([P, C], F32, tag="n2x")
        nc.vector.tensor_mul(xx[:], Xg[:], Xg[:])
        nc.vector.tensor_reduce(n2x[:], xx[:], axis=AXL.X, op=ALU.add)
        in2 = spool.tile([P, C], F32, tag="in2")
        nc.vector.reciprocal(in2[:], n2x[:])
        r0 = spool.tile([P, C], F32, tag="r0")
        nc.scalar.activation(r0[:], in2[:], ACT.Sqrt)
        t1 = spool.tile([P, C], F32, tag="t1")
        nc.vector.tensor_mul(t1[:], r0[:], r0[:])
        t2 = spool.tile([P, C], F32, tag="t2")
        nc.vector.tensor_scalar(t2[:], n2x[:], -0.5, None, op0=ALU.mult)
        nc.vector.tensor_mul(t1[:], t1[:], t2[:])
        nc.vector.tensor_scalar(t1[:], t1[:], 1.5, None, op0=ALU.add)
        invn = spool.tile([P, C], F32, tag="invn")
        nc.vector.tensor_mul(invn[:], r0[:], t1[:])

        # final rep coefficients (normalized)
        nc.vector.tensor_mul(
            ALLC[:, g * C:(g + 1) * C, :], cg[:].transpose([0, 2, 1]),
            invn[:].unsqueeze(2).to_broadcast([P, C, L]))

        # recon = sum_m ytil_m u_m ; zdl = recon / ||x||
        scr2 = spool.tile([P, C, 64, L], F32, tag="scr2")
        nc.vector.tensor_mul(
            scr2[:], Ug[:].transpose([0, 2, 3, 1]),
            YTg[:].transpose([0, 2, 1]).unsqueeze(2).to_broadcast([P, C, 64, L]))
        recon = spool.tile([P, C, 64], F32, tag="proj")
        nc.vector.tensor_reduce(recon[:], scr2[:], axis=AXL.X, op=ALU.add)

        zdl = spool.tile([P, C, 64], F32, tag="zdl")
        nc.vector.tensor_mul(zdl[:], recon[:],
                             invn[:].unsqueeze(2).to_broadcast([P, C, 64]))
        zT = ps_t.tile([64, C * P], F32, tag="tps")
        for i in range(C):
            nc.tensor.transpose(zT[:, i * P:(i + 1) * P], zdl[:, i, :],
                                IDF[:, :])
        zTs = spool.tile([64, C * P], F32, tag="zTs")
        nc.scalar.copy(zTs[:], zT[:])
        nc.sync.dma_start(zst_ap[b, :, off:off + C * P], zTs[:])

        # mse partial: ||recon - x||^2 / ||x||^2 per sample, accumulated
        diff = spool.tile([P, C, 64], F32, tag="uscr")
        nc.vector.tensor_sub(diff[:], recon[:], Xg[:])
        sq = spool.tile([P, C, 64], F32, tag="zdl")
        nc.vector.tensor_mul(sq[:], diff[:], diff[:])
        e2 = spool.tile([P, C], F32, tag="n2x")
        nc.vector.tensor_reduce(e2[:], sq[:], axis=AXL.X, op=ALU.add)
        nc.vector.tensor_mul(t1[:], invn[:], invn[:])
        nc.vector.tensor_mul(e2[:], e2[:], t1[:])
        nc.vector.tensor_add(MSEA[:], MSEA[:], e2[:])

    # ================= phase 2: rep assembly =================
    lib2 = nc.gpsimd.load_library(library_config.local_scatter)
    for g_inst in gathers:
        bass._add_dep_helper(lib2.ins, g_inst.ins, sync=True,
                             reason="lib switch after gathers")

    # reuse phase-1 psum slots (phases are sequential) for the accumulators
    cols = ps_a.tile([1, K], F32, tag="alpha")
    msep = ps_d.tile([1, 8], F32, tag="dj")

    for tt in range(nt):
        chi = spool.tile([P, L], BF16, tag="chi")
        nc.vector.tensor_copy(chi[:], ALLC[:, tt, :])
        clo32 = spool.tile([P, L], F32, tag="clo32")
        nc.vector.tensor_sub(clo32[:], ALLC[:, tt, :], chi[:])
        clo = spool.tile([P, L], BF16, tag="clo")
        nc.vector.tensor_copy(clo[:], clo32[:])

        rhi = spool.tile([P, K], BF16, tag="rhi")
        rlo = spool.tile([P, K], BF16, tag="rlo")
        s1 = nc.gpsimd.local_scatter(rhi[:], chi[:],
                                     ALLJ[:, tt, :].bitcast(I16),
                                     channels=P, num_elems=K, num_idxs=L)
        s2 = nc.gpsimd.local_scatter(rlo[:], clo[:],
                                     ALLJ[:, tt, :].bitcast(I16),
                                     channels=P, num_elems=K, num_idxs=L)
        for s in (s1, s2):
            bass._add_dep_helper(s.ins, lib2.ins, sync=True,
                                 reason="scatter after lib switch")
        repf = spool.tile([P, K], F32, tag="repf")
        nc.vector.tensor_add(repf[:], rhi[:], rlo[:])
        nc.tensor.matmul(cols[:], mmap(ONES[:]), mmap(repf[:]),
                         start=(tt == 0), stop=(tt == nt - 1),
                         skip_group_check=True)
        nc.sync.dma_start(rep_ap[tt * P:(tt + 1) * P, :], repf[:])

    # ---- partials out: [0:K] = rep column sums, [K] = sum e2n ----
    msev = spool.tile([P, 1], F32, tag="msev")
    nc.vector.tensor_reduce(msev[:], MSEA[:], axis=AXL.X, op=ALU.add)
    nc.tensor.matmul(msep[:, 0:1], mmap(ONES[:]), mmap(msev[:]),
                     start=True, stop=True, skip_group_check=True)
    parts = spool.tile([1, K + 8], F32, tag="parts")
    nc.vector.memset(parts[:], 0.0)
    nc.scalar.copy(parts[:, 0:K], cols[:])
    nc.scalar.copy(parts[:, K:K + 1], msep[:, 0:1])
    nc.sync.dma_start(part_ap[:, :], parts[:])

    ctx.close()


def _wsel_host():
    w = np.zeros((P, 8, 64), np.float32)
    for s in range(8):
        for p in range(64):
            w[16 * s + (p % 16), s, p] = 1.0
    return np.ascontiguousarray(w.reshape(P, 8 * 64))


_CACHE = {}


def build_program(nt=NT_FULL, use_f32r=False):
    key = (nt, use_f32r)
    if key in _CACHE:
        return _CACHE[key]
    nc = bacc.Bacc("TRN2", debug=False)
    z_in = nc.dram_tensor("z", (BPC, CH, HW), F32, kind="ExternalInput")
    dt_in = nc.dram_tensor("dt", (64, K), F32, kind="ExternalInput")
    idf_in = nc.dram_tensor("idf", (P, P), F32, kind="ExternalInput")
    wsel_in = nc.dram_tensor("wsel", (P, 8 * 64), F32, kind="ExternalInput")
    rep_out = nc.dram_tensor("rep", (NPC, K), F32, kind="ExternalOutput")
    zst_out = nc.dram_tensor("zst", (BPC, CH, HW), F32, kind="ExternalOutput")
    part_out = nc.dram_tensor("part", (1, K + 8), F32, kind="ExternalOutput")
    with tile.TileContext(nc) as tc:
        _build_kernel(tc, z_in.ap(), dt_in.ap(), idf_in.ap(), wsel_in.ap(),
                      rep_out.ap(), zst_out.ap(), part_out.ap(), nt, use_f32r)
    nc.compile()
    _CACHE[key] = nc
    return nc


def kernel(z_e, dictionary, use_f32r=False, _return_results=False,
           _trace=False):
    z_e = np.ascontiguousarray(z_e, dtype=np.float32)
    dictionary = np.ascontiguousarray(dictionary, dtype=np.float32)
    nc = build_program(NT_FULL, use_f32r)

    dt_host = np.ascontiguousarray(dictionary.T)           # (64, 512)
    idf = np.eye(P, dtype=np.float32)
    wsel = _wsel_host()
    zr = z_e.reshape(B, CH, HW)
    in_maps = []
    for c in range(NCORES):
        in_maps.append({
            "z": np.ascontiguousarray(zr[c * BPC:(c + 1) * BPC]),
            "dt": dt_host,
            "idf": idf,
            "wsel": wsel,
        })
    res = run_bass_kernel_spmd(nc, in_maps, list(range(NCORES)),
                               trace=_trace)

    rep = np.concatenate([r["rep"] for r in res.results], axis=0)
    zst = np.concatenate([r["zst"] for r in res.results], axis=0)
    zst = zst.reshape(B, CH, 32, 32)
    parts = np.stack([r["part"][0] for r in res.results], axis=0)  # (8, 520)

    f32 = np.float32
    colsum = parts[:, 0:K].sum(axis=0, dtype=np.float32)
    mse_num = parts[:, K].sum(dtype=np.float32)
    N = B * HW
    mse = f32(mse_num) / f32(N * CH)
    recon_loss = f32(f32(1.0 + BETA) * mse)
    avg = (colsum / f32(N)).astype(f32)
    avg = (avg / avg.sum(dtype=f32)).astype(f32)
    perplexity = f32(np.exp(-np.sum(avg * np.log(avg + f32(EPS)), dtype=f32)))
    out = (recon_loss, zst, perplexity, rep)
    if _return_results:
        return out, res
    return out


# revision 41
# speedup vs baseline: 1804.1841x; 1804.1841x over previous
"""Trainium2 Bass kernel for DictionaryLearning-kSVD (OMP sparse coding / VQ).

Contract: kernel(**inputs) takes FULL inputs (z_e (64,64,32,32) f32,
dictionary (512,64) f32) and returns the FULL outputs matching reference():
(recon_loss, z_st, perplexity, rep).

Strategy: data-parallel over the flattened sample axis N = 64*32*32 = 65536
across 8 NeuronCores (8192 samples/core).  Per core, OMP runs as a
lockstep-batched MGS (modified Gram-Schmidt) recurrence:

  alpha lives in PSUM, updated by accumulating matmuls
      alpha <- alpha - (ytil * u_t) @ D^T
  selection per step via DVE tensor_tensor_reduce (abs-max) + max_index
  dictionary row gather via GPSIMD ap_gather on the transposed dictionary
  final coefficients via batched back-substitution of the unit-triangular
  MGS basis matrix; rep rows assembled with GPSIMD local_scatter (hi/lo
  bf16 split for exact f32).

Self-contained: hardcodes all shapes; no sibling imports.
"""

import numpy as np

import concourse.bass as bass
import concourse.bacc as bacc
import concourse.mybir as mybir
import concourse.tile as tile
from concourse import library_config
from concourse.bass_utils import run_bass_kernel_spmd

F32 = mybir.dt.float32
F32R = mybir.dt.float32r
U16 = mybir.dt.uint16
I16 = mybir.dt.int16
BF16 = mybir.dt.bfloat16
ALU = mybir.AluOpType
AXL = mybir.AxisListType
ACT = mybir.ActivationFunctionType

B, CH, HW = 64, 64, 1024          # z_e: (B, CH, 32, 32) -> HW = 1024
K, L, P = 512, 8, 128             # atoms, sparsity, partition tile
NCORES = 8
BPC = B // NCORES                 # 8 batches per core
NPC = BPC * HW                    # 8192 samples per core
C = 4                             # tiles per lockstep group (default)
NT_FULL = NPC // P                # 64 tiles per core
BETA, EPS = 0.25, 1e-10


def _build_kernel(tc, z_ap, dt_ap, idf_ap, wsel_ap, rep_ap, zst_ap, part_ap,
                  nt, use_f32r, C=C, abufs=1, sbufs=2, gbufs=2, tbufs=3,
                  dbufs=1, asbuf=False, pair=2, vsplit=False):
    nc = tc.nc
    ngrp = nt // C
    mmdt = F32R if use_f32r else F32

    def mmap(ap):
        return ap.bitcast(F32R) if use_f32r else ap

    from contextlib import ExitStack
    ctx = ExitStack()
    const = ctx.enter_context(tc.tile_pool(name="const", bufs=1))
    gpool = ctx.enter_context(tc.tile_pool(name="grp", bufs=gbufs))
    spool = ctx.enter_context(tc.tile_pool(name="step", bufs=sbufs))
    accp = ctx.enter_context(tc.tile_pool(name="acc", bufs=1))
    ps_a = ctx.enter_context(tc.tile_pool(name="psA", bufs=abufs, space="PSUM"))
    ps_t = ctx.enter_context(tc.tile_pool(name="psT", bufs=tbufs, space="PSUM"))
    ps_d = ctx.enter_context(tc.tile_pool(name="psD", bufs=dbufs, space="PSUM"))

    # ---- constants ----
    DT = const.tile([64, K], F32)           # dictionary^T
    nc.sync.dma_start(DT[:], dt_ap[:, :])
    IDF = const.tile([P, P], F32)           # identity for PE transposes
    nc.sync.dma_start(IDF[:], idf_ap[:, :])
    WSEL = const.tile([P, 8 * 64], F32)     # wrap-selector matrices
    nc.sync.dma_start(WSEL[:], wsel_ap[:, :])
    ONES = const.tile([P, 1], F32)
    nc.vector.memset(ONES[:], 1.0)
    ZERO = const.tile([P, 1], F32)
    nc.vector.memset(ZERO[:], 0.0)

    ALLJF = accp.tile([P, nt, L], F32)
    ALLC = accp.tile([P, nt, L], F32)
    MSEA = accp.tile([P, C], F32)
    nc.vector.memset(MSEA[:], 0.0)

    lib1 = nc.gpsimd.load_library(library_config.ap_gather)
    gathers = []

    # ================= phase 1: selection + solve =================
    # Two group-chains are emitted step-interleaved so the Tile scheduler can
    # fill one chain's cross-engine latency with the other chain's DVE work.

    def grp_head(g):
        gpb = HW // (C * P)            # groups per batch
        b, half = g // gpb, g % gpb
        off = half * (C * P)

        XT = gpool.tile([64, C * P], F32, tag="XT")
        nc.sync.dma_start(XT[:], z_ap[b, :, off:off + C * P])

        # sample-major X via PE transposes
        xps = ps_t.tile([P, C * 64], F32, tag="tps")
        for i in range(C):
            nc.tensor.transpose(xps[:, i * 64:(i + 1) * 64],
                                XT[:, i * P:(i + 1) * P], IDF[:64, :64])
        Xg = gpool.tile([P, C, 64], F32, tag="Xg")
        nc.scalar.copy(Xg[:], xps[:].rearrange("p (c d) -> p c d", c=C))

        # alpha0 = X @ D^T
        if asbuf:
            alpha = gpool.tile([P, C * K], F32, tag="alphas")
            H = C // 2 if vsplit else C
            for h0 in range(0, C, H):
                vps = ps_a.tile([P, H * K], F32, tag="alpha")
                for ii in range(H):
                    i = h0 + ii
                    nc.tensor.matmul(vps[:, ii * K:(ii + 1) * K],
                                     mmap(XT[:, i * P:(i + 1) * P]),
                                     mmap(DT[:]), start=True, stop=True,
                                     skip_group_check=True)
                nc.scalar.copy(alpha[:, h0 * K:(h0 + H) * K], vps[:])
        else:
            alpha = ps_a.tile([P, C * K], F32, tag="alpha")
            for i in range(C):
                nc.tensor.matmul(alpha[:, i * K:(i + 1) * K],
                                 mmap(XT[:, i * P:(i + 1) * P]), mmap(DT[:]),
                                 start=True, stop=False, skip_group_check=True)

        Ug = gpool.tile([P, L, C, 64], F32, tag="Ug")
        Bg = gpool.tile([P, L, L, C], F32, tag="Bg")
        NUg = gpool.tile([P, L, C], F32, tag="NUg")
        YTg = gpool.tile([P, L, C], F32, tag="YTg")
        return dict(g=g, b=b, off=off, XT=XT, Xg=Xg, alpha=alpha,
                    Ug=Ug, Bg=Bg, NUg=NUg, YTg=YTg)

    def grp_step(st, t):
        g, alpha, Xg = st["g"], st["alpha"], st["Xg"]
        Ug, Bg, NUg, YTg = st["Ug"], st["Bg"], st["NUg"], st["YTg"]
        # -- selection --
        absa = spool.tile([P, C, K], F32, tag="absa")
        vmax = spool.tile([P, C], F32, tag="vmax")
        for i in range(C):
            nc.scalar.activation(absa[:, i, :],
                                 alpha[:, i * K:(i + 1) * K], ACT.Abs)
        # abs-max straight from alpha — concurrent with the ScalarE Abs pass
        nc.vector.tensor_reduce(
            vmax[:], alpha[:].rearrange("p (c k) -> p c k", c=C),
            axis=AXL.X, op=ALU.max, apply_absolute_value=True)
        idx8 = spool.tile([P, C, 8], U16, tag="idx8")
        for i in range(C):
            nc.vector.max_index(idx8[:, i, :],
                                vmax[:, i:i + 1].to_broadcast([P, 8]),
                                absa[:, i, :])
        jf = ALLJF[:, g * C:(g + 1) * C, t]
        nc.vector.tensor_copy(jf[:], idx8[:, :, 0])

        # -- index wrap for ap_gather via constant selector matmuls:
        #    idxw[16g+q, c, s] = j[16s+q, c]
        jwps = ps_d.tile([64, 8, C], F32, tag="dj")
        for s in range(8):
            nc.tensor.matmul(jwps[:, s, :],
                             WSEL[:, s * 64:(s + 1) * 64], jf[:],
                             start=True, stop=True, skip_group_check=True)
        idxw = spool.tile([64, C, 8], I16, tag="idxw")
        nc.scalar.copy(idxw[:], jwps[:].transpose([0, 2, 1]))

        # -- gather dictionary rows (transposed): djT[d, s] = D^T[d, j_s]
        djT = spool.tile([64, C, P], F32, tag="djT")
        g_inst = nc.gpsimd.ap_gather(
            out_ap=djT[:].rearrange("p c s -> p (c s)").unsqueeze(2),
            in_ap=DT[:].unsqueeze(2),
            idxs_ap=idxw[:].rearrange("p c s -> p (c s)"),
            channels=64, num_elems=K, d=1, num_idxs=C * P)
        bass._add_dep_helper(g_inst.ins, lib1.ins, sync=True,
                             reason="gather after lib1")
        gathers.append(g_inst)
        dj = ps_d.tile([P, C, 64], F32, tag="dj")
        for i in range(C):
            nc.tensor.transpose(dj[:, i, :], djT[:, i, :], IDF[:64, :64])

        # -- MGS orthogonalization --
        if t > 0:
            scr1 = spool.tile([P, L, C, 64], F32, tag="scr2")
            nc.vector.tensor_mul(
                scr1[:, :t, :, :], Ug[:, :t, :, :],
                dj[:].unsqueeze(1).to_broadcast([P, t, C, 64]))
            wtil = spool.tile([P, L, C], F32, tag="wtil")
            nc.vector.tensor_reduce(wtil[:, :t, :], scr1[:, :t, :, :],
                                    axis=AXL.X, op=ALU.add)
            wbar = Bg[:, t, :t, :]
            nc.vector.tensor_mul(wbar, wtil[:, :t, :], NUg[:, :t, :])
            scr2 = spool.tile([P, C, 64, L], F32, tag="scr2")
            nc.vector.tensor_mul(
                scr2[:, :, :, :t],
                Ug[:, :t, :, :].transpose([0, 2, 3, 1]),
                wbar.transpose([0, 2, 1]).unsqueeze(2)
                    .to_broadcast([P, C, 64, t]))
            proj = spool.tile([P, C, 64], F32, tag="proj")
            nc.vector.tensor_reduce(proj[:], scr2[:, :, :, :t],
                                    axis=AXL.X, op=ALU.add)
            nc.vector.tensor_sub(Ug[:, t, :, :], dj[:], proj[:])
        else:
            nc.vector.tensor_copy(Ug[:, 0, :, :], dj[:])

        uscr = spool.tile([P, C, 64], F32, tag="uscr")
        n2v = spool.tile([P, C], F32, tag="n2v")
        nc.vector.tensor_mul(uscr[:], Ug[:, t, :, :], Ug[:, t, :, :])
        nc.vector.tensor_reduce(n2v[:], uscr[:], axis=AXL.X, op=ALU.add)
        yv = spool.tile([P, C], F32, tag="yv")
        nc.vector.tensor_mul(uscr[:], Ug[:, t, :, :], Xg[:])
        nc.vector.tensor_reduce(yv[:], uscr[:], axis=AXL.X, op=ALU.add)
        nc.vector.reciprocal(NUg[:, t, :], n2v[:])
        nc.vector.tensor_mul(YTg[:, t, :], yv[:], NUg[:, t, :])

        # -- alpha update: alpha -= ytil * (u @ D^T) --
        utl = spool.tile([P, C, 64], F32, tag="utl")
        nc.vector.scalar_tensor_tensor(
            utl[:], Ug[:, t, :, :], -1.0,
            YTg[:, t, :].unsqueeze(2).to_broadcast([P, C, 64]),
            op0=ALU.mult, op1=ALU.mult)
        uT = ps_t.tile([64, C * P], F32, tag="tps")
        for i in range(C):
            nc.tensor.transpose(uT[:, i * P:(i + 1) * P], utl[:, i, :],
                                IDF[:, :])
        uTs = spool.tile([64, C * P], F32, tag="uTs")
        nc.scalar.copy(uTs[:], uT[:])
        if asbuf:
            H = C // 2 if vsplit else C
            for h0 in range(0, C, H):
                vps = ps_a.tile([P, H * K], F32, tag="alpha")
                for ii in range(H):
                    i = h0 + ii
                    nc.tensor.matmul(vps[:, ii * K:(ii + 1) * K],
                                     mmap(uTs[:, i * P:(i + 1) * P]),
                                     mmap(DT[:]), start=True, stop=True,
                                     skip_group_check=True)
                nc.vector.tensor_add(alpha[:, h0 * K:(h0 + H) * K],
                                     alpha[:, h0 * K:(h0 + H) * K], vps[:])
        else:
            for i in range(C):
                nc.tensor.matmul(alpha[:, i * K:(i + 1) * K],
                                 mmap(uTs[:, i * P:(i + 1) * P]), mmap(DT[:]),
                                 start=False, stop=(t == L - 1),
                                 skip_group_check=True)

    def grp_tail(st):
        g, b, off, Xg = st["g"], st["b"], st["off"], st["Xg"]
        Ug, Bg, NUg, YTg = st["Ug"], st["Bg"], st["NUg"], st["YTg"]
        # backsolve B c = ytil
        cg = gpool.tile([P, L, C], F32, tag="cg")
        nc.vector.tensor_copy(cg[:], YTg[:])
        bscr = spool.tile([P, C, L], F32, tag="bscr")
        red = spool.tile([P, C], F32, tag="red")
        for tt in range(L - 2, -1, -1):
            n = L - 1 - tt
            nc.vector.tensor_mul(bscr[:, :, :n],
                                 Bg[:, tt + 1:, tt, :].transpose([0, 2, 1]),
                                 cg[:, tt + 1:, :].transpose([0, 2, 1]))
            nc.vector.tensor_reduce(red[:], bscr[:, :, :n], axis=AXL.X,
                                    op=ALU.add)
            nc.vector.tensor_sub(cg[:, tt, :], cg[:, tt, :],
                                 red[:].rearrange("p c -> p c"))

        # 1/||x|| with one Newton refinement
        xx = spool.tile([P, C, 64], F32, tag="uscr")
        n2x = spool.tile([P, C], F32, tag="n2x")
        nc.vector.tensor_mul(xx[:], Xg[:], Xg[:])
        nc.vector.tensor_reduce(n2x[:], xx[:], axis=AXL.X, op=ALU.add)
        in2 = spool.tile([P, C], F32, tag="in2")
        nc.vector.reciprocal(in2[:], n2x[:])
        r0 = spool.tile([P, C], F32, tag="r0")
        nc.scalar.activation(r0[:], in2[:], ACT.Sqrt)
        t1 = spool.tile([P, C], F32, tag="t1")
        nc.vector.tensor_mul(t1[:], r0[:], r0[:])
        t2 = spool.tile([P, C], F32, tag="t2")
        nc.vector.tensor_scalar(t2[:], n2x[:], -0.5, None, op0=ALU.mult)
        nc.vector.tensor_mul(t1[:], t1[:], t2[:])
        nc.vector.tensor_scalar(t1[:], t1[:], 1.5, None, op0=ALU.add)
        invn = spool.tile([P, C], F32, tag="invn")
        nc.vector.tensor_mul(invn[:], r0[:], t1[:])

        # final rep coefficients (normalized)
        nc.vector.tensor_mul(
            ALLC[:, g * C:(g + 1) * C, :], cg[:].transpose([0, 2, 1]),
            invn[:].unsqueeze(2).to_broadcast([P, C, L]))

        # recon = sum_m ytil_m u_m ; zdl = recon / ||x||
        scr2 = spool.tile([P, C, 64, L], F32, tag="scr2")
        nc.vector.tensor_mul(
            scr2[:], Ug[:].transpose([0, 2, 3, 1]),
            YTg[:].transpose([0, 2, 1]).unsqueeze(2).to_broadcast([P, C, 64, L]))
        recon = spool.tile([P, C, 64], F32, tag="proj")
        nc.vector.tensor_reduce(recon[:], scr2[:], axis=AXL.X, op=ALU.add)

        zdl = spool.tile([P, C, 64], F32, tag="zdl")
        nc.vector.tensor_mul(zdl[:], recon[:],
                             invn[:].unsqueeze(2).to_broadcast([P, C, 64]))
        zT = ps_t.tile([64, C * P], F32, tag="tps")
        for i in range(C):
            nc.tensor.transpose(zT[:, i * P:(i + 1) * P], zdl[:, i, :],
                                IDF[:, :])
        zTs = spool.tile([64, C * P], F32, tag="zTs")
        nc.scalar.copy(zTs[:], zT[:])
        nc.sync.dma_start(zst_ap[b, :, off:off + C * P], zTs[:])

        # mse partial: ||recon - x||^2 / ||x||^2 per sample, accumulated
        diff = spool.tile([P, C, 64], F32, tag="uscr")
        nc.vector.tensor_sub(diff[:], recon[:], Xg[:])
        sq = spool.tile([P, C, 64], F32, tag="zdl")
        nc.vector.tensor_mul(sq[:], diff[:], diff[:])
        e2 = spool.tile([P, C], F32, tag="n2x")
        nc.vector.tensor_reduce(e2[:], sq[:], axis=AXL.X, op=ALU.add)
        nc.vector.tensor_mul(t1[:], invn[:], invn[:])
        nc.vector.tensor_mul(e2[:], e2[:], t1[:])
        nc.vector.tensor_add(MSEA[:], MSEA[:], e2[:])

    PAIR = pair
    for g0 in range(0, ngrp, PAIR):
        sts = [grp_head(g0 + d) for d in range(min(PAIR, ngrp - g0))]
        for t in range(L):
            for st in sts:
                grp_step(st, t)
        for st in sts:
            grp_tail(st)

    # ================= phase 2: rep assembly =================
    lib2 = nc.gpsimd.load_library(library_config.local_scatter)
    for g_inst in gathers:
        bass._add_dep_helper(lib2.ins, g_inst.ins, sync=True,
                             reason="lib switch after gathers")

    # reuse phase-1 psum slots (phases are sequential) for the accumulators
    cols = ps_a.tile([1, K], F32, tag="alpha")
    msep = ps_d.tile([1, 8], F32, tag="dj")

    for tt in range(nt):
        chi = spool.tile([P, L], BF16, tag="chi")
        nc.vector.tensor_copy(chi[:], ALLC[:, tt, :])
        clo32 = spool.tile([P, L], F32, tag="clo32")
        nc.vector.tensor_sub(clo32[:], ALLC[:, tt, :], chi[:])
        clo = spool.tile([P, L], BF16, tag="clo")
        nc.vector.tensor_copy(clo[:], clo32[:])

        ji = spool.tile([P, L], I16, tag="ji")
        nc.vector.tensor_copy(ji[:], ALLJF[:, tt, :])
        rhi = spool.tile([P, K], BF16, tag="rhi")
        rlo = spool.tile([P, K], BF16, tag="rlo")
        s1 = nc.gpsimd.local_scatter(rhi[:], chi[:], ji[:],
                                     channels=P, num_elems=K, num_idxs=L)
        s2 = nc.gpsimd.local_scatter(rlo[:], clo[:], ji[:],
                                     channels=P, num_elems=K, num_idxs=L)
        for s in (s1, s2):
            bass._add_dep_helper(s.ins, lib2.ins, sync=True,
                                 reason="scatter after lib switch")
        repf = spool.tile([P, K], F32, tag="repf")
        nc.vector.tensor_add(repf[:], rhi[:], rlo[:])
        nc.tensor.matmul(cols[:], mmap(ONES[:]), mmap(repf[:]),
                         start=(tt == 0), stop=(tt == nt - 1),
                         skip_group_check=True)
        nc.sync.dma_start(rep_ap[tt * P:(tt + 1) * P, :], repf[:])

    # ---- partials out: [0:K] = rep column sums, [K] = sum e2n ----
    msev = spool.tile([P, 1], F32, tag="msev")
    nc.vector.tensor_reduce(msev[:], MSEA[:], axis=AXL.X, op=ALU.add)
    nc.tensor.matmul(msep[:, 0:1], mmap(ONES[:]), mmap(msev[:]),
                     start=True, stop=True, skip_group_check=True)
    parts = spool.tile([1, K + 8], F32, tag="parts")
    nc.vector.memset(parts[:], 0.0)
    nc.scalar.copy(parts[:, 0:K], cols[:])
    nc.scalar.copy(parts[:, K:K + 1], msep[:, 0:1])
    nc.sync.dma_start(part_ap[:, :], parts[:])

    ctx.close()


def _wsel_host():
    w = np.zeros((P, 8, 64), np.float32)
    for s in range(8):
        for p in range(64):
            w[16 * s + (p % 16), s, p] = 1.0
    return np.ascontiguousarray(w.reshape(P, 8 * 64))


_CACHE = {}


BEST_CFG = dict(asbuf=True, abufs=1, sbufs=3, gbufs=3, tbufs=3, dbufs=3,
                pair=2, vsplit=True)


def build_program(nt=NT_FULL, use_f32r=False, **kw):
    kw = {**BEST_CFG, **kw}
    key = (nt, use_f32r, tuple(sorted(kw.items())))
    if key in _CACHE:
        return _CACHE[key]
    nc = bacc.Bacc("TRN2", debug=False)
    z_in = nc.dram_tensor("z", (BPC, CH, HW), F32, kind="ExternalInput")
    dt_in = nc.dram_tensor("dt", (64, K), F32, kind="ExternalInput")
    idf_in = nc.dram_tensor("idf", (P, P), F32, kind="ExternalInput")
    wsel_in = nc.dram_tensor("wsel", (P, 8 * 64), F32, kind="ExternalInput")
    rep_out = nc.dram_tensor("rep", (NPC, K), F32, kind="ExternalOutput")
    zst_out = nc.dram_tensor("zst", (BPC, CH, HW), F32, kind="ExternalOutput")
    part_out = nc.dram_tensor("part", (1, K + 8), F32, kind="ExternalOutput")
    with tile.TileContext(nc) as tc:
        _build_kernel(tc, z_in.ap(), dt_in.ap(), idf_in.ap(), wsel_in.ap(),
                      rep_out.ap(), zst_out.ap(), part_out.ap(), nt, use_f32r,
                      **kw)
    nc.compile()
    _CACHE[key] = nc
    return nc


def kernel(z_e, dictionary, use_f32r=False, _return_results=False,
           _trace=False):
    z_e = np.ascontiguousarray(z_e, dtype=np.float32)
    dictionary = np.ascontiguousarray(dictionary, dtype=np.float32)
    nc = build_program(NT_FULL, use_f32r)

    dt_host = np.ascontiguousarray(dictionary.T)           # (64, 512)
    idf = np.eye(P, dtype=np.float32)
    wsel = _wsel_host()
    zr = z_e.reshape(B, CH, HW)
    in_maps = []
    for c in range(NCORES):
        in_maps.append({
            "z": np.ascontiguousarray(zr[c * BPC:(c + 1) * BPC]),
            "dt": dt_host,
            "idf": idf,
            "wsel": wsel,
        })
    res = run_bass_kernel_spmd(nc, in_maps, list(range(NCORES)),
                               trace=_trace)

    rep = np.concatenate([r["rep"] for r in res.results], axis=0)
    zst = np.concatenate([r["zst"] for r in res.results], axis=0)
    zst = zst.reshape(B, CH, 32, 32)
    parts = np.stack([r["part"][0] for r in res.results], axis=0)  # (8, 520)

    f32 = np.float32
    colsum = parts[:, 0:K].sum(axis=0, dtype=np.float32)
    mse_num = parts[:, K].sum(dtype=np.float32)
    N = B * HW
    mse = f32(mse_num) / f32(N * CH)
    recon_loss = f32(f32(1.0 + BETA) * mse)
    avg = (colsum / f32(N)).astype(f32)
    avg = (avg / avg.sum(dtype=f32)).astype(f32)
    perplexity = f32(np.exp(-np.sum(avg * np.log(avg + f32(EPS)), dtype=f32)))
    out = (recon_loss, zst, perplexity, rep)
    if _return_results:
        return out, res
    return out
